# revision 1
# baseline (speedup 1.0000x reference)
"""Trainium2 Bass kernel for nn_PositionalScore.

Math (L=8192, D=64, T=9, P=131072, Q=65536):
  out = sum_t sum_p emb[i_tp] @ W_t @ emb[j_tp]  + P * sum(b)
        + 7 clamped-table-lookup sums over Q indices each.

Strategy (8-way data parallel over pairs / table indices):
  - Pair bilinear term: sum_p e_i W_t e_j = <sum_p e_i (x) e_j, W_t>_F.
    Each core gathers its 2*16384 embedding rows per t via SWDGE dma_gather
    (256B rows), PE accumulates G_t = sum_p outer(e_i, e_j) in PSUM via
    128-pair matmuls (lhsT=Ei [128,64], rhs=Ej [128,64]), then DVE takes the
    Frobenius inner product with W_t.
  - Table terms: DVE builds per-partition histograms of the 8192 local
    indices per table (is_equal per bin, is_ge for the clamp bin) and dots
    them with the table values; the b-term is folded in as a constant
    histogram column.
  - gpsimd partition_all_reduce -> one f32 scalar per core; host sums 8.
"""

import numpy as np

import concourse.bass as bass  # noqa: F401  (registers engine classes)
import concourse.bacc as bacc
from concourse import mybir, bass_isa
from concourse.bass_utils import run_bass_kernel_spmd
from concourse.library_config import mlp

L, D, T, P, Q = 8192, 64, 9, 131072, 65536
N_CORES = 8
PC = P // N_CORES          # pairs per core per t
QC = Q // N_CORES          # table idxs per core per table
BATCH_IDXS = 1024          # gathered rows per dma_gather (HW fails >= 8192)
NB = T * (2 * PC) // BATCH_IDXS   # gather batches per core
IDX_COLS = NB * (BATCH_IDXS // 16)  # 18432 int16 idx columns
CPB = BATCH_IDXS // 16     # idx columns per batch
EBC = BATCH_IDXS // 128    # embedding-buffer columns per batch
MPB = EBC // 2             # matmuls per batch
BPT = NB // T              # batches per t slice

_NC_CACHE = {}


def build_program(reps: int = 1):
    A = mybir.AluOpType
    nc = bacc.Bacc("TRN2", target_bir_lowering=False, debug=False,
                   num_devices=N_CORES, num_swdge_queues=4)
    emb_d = nc.dram_tensor("emb", [L, D], mybir.dt.float32, kind="ExternalInput")
    gidx_d = nc.dram_tensor("gidx", [128, IDX_COLS], mybir.dt.int16,
                            kind="ExternalInput")
    tabidx_d = nc.dram_tensor("tabidx", [128, 512], mybir.dt.int32,
                              kind="ExternalInput")
    wsb_d = nc.dram_tensor("wsb", [64, T * 64], mybir.dt.float32,
                           kind="ExternalInput")
    tabs_d = nc.dram_tensor("tabs", [128, 240], mybir.dt.float32,
                            kind="ExternalInput")
    out_d = nc.dram_tensor("out", [1, 1], mybir.dt.float32,
                           kind="ExternalOutput")

    from contextlib import ExitStack
    with ExitStack() as stack, nc.Block() as block:
        ec = stack.enter_context
        gidx_s = ec(nc.sbuf_tensor("gidx_s", [128, IDX_COLS], mybir.dt.int16))
        eb0 = ec(nc.sbuf_tensor("eb0", [128, EBC, 64], mybir.dt.float32))
        eb1 = ec(nc.sbuf_tensor("eb1", [128, EBC, 64], mybir.dt.float32))
        eb2 = ec(nc.sbuf_tensor("eb2", [128, EBC, 64], mybir.dt.float32))
        tabidx_s = ec(nc.sbuf_tensor("tabidx_s", [128, 512], mybir.dt.int32))
        idxf = ec(nc.sbuf_tensor("idxf", [128, 512], mybir.dt.float32))
        scr = ec(nc.sbuf_tensor("scr", [128, 64], mybir.dt.float32))
        e0c = ec(nc.sbuf_tensor("e0c", [128, 64], mybir.dt.float32))
        comb = ec(nc.sbuf_tensor("comb", [128, 64], mybir.dt.float32))
        cnt = ec(nc.sbuf_tensor("cnt", [128, 240], mybir.dt.float32))
        tabs_s = ec(nc.sbuf_tensor("tabs_s", [128, 240], mybir.dt.float32))
        ttrash = ec(nc.sbuf_tensor("ttrash", [128, 240], mybir.dt.float32))
        wsb_s = ec(nc.sbuf_tensor("wsb_s", [64, T * 64], mybir.dt.float32))
        prod = ec(nc.sbuf_tensor("prod", [64, T * 64], mybir.dt.float32))
        tab_e = ec(nc.sbuf_tensor("tab_e", [128, 1], mybir.dt.float32))
        bil_e = ec(nc.sbuf_tensor("bil_e", [64, 1], mybir.dt.float32))
        red = ec(nc.sbuf_tensor("red", [128, 1], mybir.dt.float32))
        Sa = ec(nc.psum_tensor("Sa", [64, 512], mybir.dt.float32))
        Sb = ec(nc.psum_tensor("Sb", [64, 64], mybir.dt.float32))
        io = ec(nc.semaphore("io"))
        gsems = [ec(nc.semaphore(f"gsem{i}")) for i in range(3)]
        psem = ec(nc.semaphore("psem"))
        dsem = ec(nc.semaphore("dsem"))
        vsem = ec(nc.semaphore("vsem"))
        rsem = ec(nc.semaphore("rsem"))
        ebufs = [eb0, eb1, eb2]

        @block.sync
        def _(sync):
            sync.dma_start(gidx_s[:], gidx_d[:]).then_inc(io, 16)
            sync.dma_start(tabidx_s[:], tabidx_d[:]).then_inc(io, 16)
            sync.dma_start(wsb_s[:], wsb_d[:]).then_inc(io, 16)
            sync.dma_start(tabs_s[:], tabs_d[:]).then_inc(io, 16)
            for r in range(reps):
                sync.wait_ge(rsem, r + 1)
                sync.wait_ge(io, 64 + 16 * r)
                sync.dma_start(out_d[:], red[0:1, :]).then_inc(io, 16)

        @block.gpsimd
        def _(g):
            g.load_library(mlp)
            g.wait_ge(io, 64)
            for r in range(reps):
                for b in range(NB):
                    gb = r * NB + b   # global batch number
                    if gb >= 3:
                        g.wait_ge(psem, gb - 2)
                        # same-sem issuer wait: orders this slot's DMA incs
                        g.wait_ge(gsems[gb % 3], 16 * (gb // 3))
                    g.dma_gather(
                        ebufs[gb % 3][:], emb_d[:],
                        gidx_s[:, b * CPB:(b + 1) * CPB],
                        BATCH_IDXS, BATCH_IDXS, D,
                        queue_num=gb % 4,
                    ).then_inc(gsems[gb % 3], 16)
                g.wait_ge(dsem, r + 1)
                if r > 0:
                    g.wait_ge(io, 64 + 16 * r)  # prior out_d DMA drained
                g.partition_all_reduce(red[:], tab_e[:], 128,
                                       bass_isa.ReduceOp.add).then_inc(rsem, 1)
            g.wait_ge(io, 64 + 16 * reps)

        @block.tensor
        def _(pe):
            for r in range(reps):
                if r > 0:
                    pe.wait_ge(dsem, r)  # DVE done reading PSUM from rep r-1
                for b in range(NB):
                    gb = r * NB + b
                    t, ph = b // BPT, b % BPT
                    pe.wait_ge(gsems[gb % 3], 16 * (gb // 3 + 1))
                    eb = ebufs[gb % 3]
                    out = Sa[:, t * 64:(t + 1) * 64] if t < 8 else Sb[:]
                    for m in range(MPB):
                        inst = pe.matmul(
                            out, eb[:, 2 * m, :], eb[:, 2 * m + 1, :],
                            start=(ph == 0 and m == 0),
                            stop=(ph == BPT - 1 and m == MPB - 1),
                        )
                    inst.then_inc(psem, 1)

        @block.vector
        def _(v):
            # The race model gives no implicit same-engine ordering, so every
            # DVE instruction is chained through vsem.
            nv = [0]

            def V(inst):
                inst.then_inc(vsem, 1)
                nv[0] += 1
                v.wait_ge(vsem, nv[0])
                return inst

            v.wait_ge(io, 64)
            for r in range(reps):
                V(v.tensor_copy(idxf[:], tabidx_s[:]))
                # zero only the padding columns; bin/b columns are overwritten
                for lo, hi in ((31, 32), (63, 64), (95, 96), (112, 128),
                               (157, 160), (191, 192), (217, 224), (233, 240)):
                    V(v.memset(cnt[:, lo:hi], 0.0))
                V(v.memset(cnt[:, 224:224 + T], 128.0))
                segs = [(0, 0, 31), (1, 32, 31), (2, 64, 31),
                        (3, 96, 16), (4, 128, 29), (5, 160, 31)]
                for s, base, nbins in segs:
                    seg = idxf[:, s * 64:(s + 1) * 64]
                    for k in range(nbins - 1):
                        V(v.tensor_scalar(scr[:], seg, float(k), 0.0,
                                          A.is_equal, A.add,
                                          accum_out=cnt[:, base + k:base + k + 1]))
                    V(v.tensor_scalar(scr[:], seg, float(nbins - 1), 0.0,
                                      A.is_ge, A.add,
                                      accum_out=cnt[:, base + nbins - 1:base + nbins]))
                # explicit: comb = min(e0,4)*5 + min(e1,4), bins 0..24
                V(v.tensor_scalar(e0c[:], idxf[:, 384:448], 4.0, 5.0,
                                  A.min, A.mult))
                V(v.tensor_scalar(comb[:], idxf[:, 448:512], 4.0, None, A.min))
                V(v.tensor_tensor(comb[:], comb[:], e0c[:], A.add))
                for k in range(25):
                    V(v.tensor_scalar(scr[:], comb[:], float(k), 0.0,
                                      A.is_equal, A.add,
                                      accum_out=cnt[:, 192 + k:192 + k + 1]))
                if r > 0:
                    v.wait_ge(rsem, r)  # gpsimd done reading tab_e of rep r-1
                V(v.tensor_tensor(ttrash[:], cnt[:], tabs_s[:], A.mult))
                V(v.tensor_scalar(ttrash[:], ttrash[:], 1.0, 0.0,
                                  A.mult, A.add, accum_out=tab_e[:]))
                v.wait_ge(psem, NB * (r + 1))
                V(v.tensor_tensor(prod[:, 0:512], Sa[:], wsb_s[:, 0:512],
                                  A.mult))
                V(v.tensor_tensor(prod[:, 512:576], Sb[:], wsb_s[:, 512:576],
                                  A.mult))
                V(v.tensor_scalar(prod[:], prod[:], 1.0, 0.0,
                                  A.mult, A.add, accum_out=bil_e[:]))
                v.tensor_tensor(tab_e[0:64, :], tab_e[0:64, :], bil_e[:],
                                A.add).then_inc(dsem, 1)
                v.wait_ge(dsem, r + 1)

    nc.compile()
    return nc


def _get_nc(reps: int = 1):
    if reps not in _NC_CACHE:
        _NC_CACHE[reps] = build_program(reps)
    return _NC_CACHE[reps]


def make_in_maps(inputs: dict) -> list[dict]:
    emb = np.ascontiguousarray(np.asarray(inputs["embedding"], np.float32))
    W = np.asarray(inputs["W"], np.float32)
    b = np.asarray(inputs["b"], np.float32)
    pair_idx = np.asarray(inputs["pair_idx"], np.int32)
    explicit = np.asarray(inputs["explicit_idx"], np.int32)

    wsb = np.ascontiguousarray(W.transpose(1, 0, 2).reshape(D, T * D))

    tabs_row = np.zeros(240, np.float32)
    tabs_row[0:31] = np.asarray(inputs["hairpin_length"], np.float32)
    tabs_row[32:63] = np.asarray(inputs["bulge_length"], np.float32)
    tabs_row[64:95] = np.asarray(inputs["internal_length"], np.float32)
    tabs_row[96:112] = np.asarray(inputs["internal_symmetry"], np.float32)
    tabs_row[128:157] = np.asarray(inputs["internal_asymmetry"], np.float32)
    tabs_row[160:191] = np.asarray(inputs["helix_length"], np.float32)
    tabs_row[192:217] = np.asarray(inputs["internal_explicit"],
                                   np.float32).reshape(25)
    tabs_row[224:233] = b
    tabs = np.ascontiguousarray(np.tile(tabs_row[None, :], (128, 1)))

    tab_arrs = [np.asarray(inputs[k], np.int32) for k in
                ("hairpin_idx", "bulge_idx", "internal_len_idx",
                 "symmetry_idx", "asymmetry_idx", "helix_idx")]

    in_maps = []
    for c in range(N_CORES):
        pi = pair_idx[:, c * PC:(c + 1) * PC, :]           # [T, PC, 2]
        flat = pi.reshape(T, PC // 128, 128, 2).transpose(0, 1, 3, 2)
        flat = flat.reshape(-1).astype(np.int16)           # [T*2*PC]
        gidx = np.ascontiguousarray(
            np.tile(flat.reshape(-1, 16).T, (8, 1)))       # [128, IDX_COLS]

        cols = [a[c * QC:(c + 1) * QC].reshape(128, 64) for a in tab_arrs]
        cols.append(explicit[c * QC:(c + 1) * QC, 0].reshape(128, 64))
        cols.append(explicit[c * QC:(c + 1) * QC, 1].reshape(128, 64))
        tabidx = np.ascontiguousarray(np.concatenate(cols, axis=1))

        in_maps.append({"emb": emb, "gidx": gidx, "tabidx": tabidx,
                        "wsb": wsb, "tabs": tabs})
    return in_maps


def run(in_maps, reps: int = 1):
    nc = _get_nc(reps)
    return run_bass_kernel_spmd(nc, in_maps, list(range(N_CORES)))


def kernel(**inputs) -> np.ndarray:
    in_maps = make_in_maps(inputs)
    res = run(in_maps, reps=1)
    total = np.float64(0.0)
    for c in range(N_CORES):
        total += np.float64(res.results[c]["out"].reshape(()))
    return np.array(total, dtype=np.float32)



# revision 2
# speedup vs baseline: 1.1616x; 1.1616x over previous
"""Trainium2 Bass kernel for nn_PositionalScore — ap_gather design.

Per core per rep (8-way data parallel over pairs / table indices):
  - E^T lives in SBUF as per-partition columns: partition p holds E[:, p%64]
    (dims replicated across the two 64-partition halves).
  - One gpsimd ap_gather per 2048-pair chunk fetches BOTH sides in [d, pair]
    layout: groups 0-3 (partitions 0-63) gather e_i columns, groups 4-7
    (partitions 64-127) gather e_j columns.
  - PE: Z = W_t^T @ Ej^T per chunk (lhsT/rhs at partition base 64,
    tile_position=(64,0)), accumulated nowhere — each chunk's Z is dotted
    immediately.
  - DVE: <Ei^T, Z> per chunk via tensor_tensor_reduce -> pacc column.
  - Score tables: clamped-index lookups via 4+1 more ap_gathers from a
    per-partition score table; index clamp/combine on DVE int16.
  - Final: per-partition partials -> ones-matmul over 128 partitions ->
    scalar; host sums the 8 per-core scalars.
"""

import numpy as np

import concourse.bass as bass  # noqa: F401
import concourse.bacc as bacc
from concourse import mybir
from concourse.bass_utils import run_bass_kernel_spmd
from concourse.library_config import ap_gather as apg_lib

L, D, T, P, Q = 8192, 64, 9, 131072, 65536
N_CORES = 8
PC = P // N_CORES            # pairs per core per t (16384)
QC = Q // N_CORES            # table idxs per core per table (8192)
NI = 2048                    # pairs per ap_gather chunk
NCH = T * PC // NI           # 72 chunks per rep (8 per t)
CPB = NI // 16               # idx cols per chunk (128)
TNI = 2048                   # lookups per group per table-gather
TIN = 4                      # table gather instructions (4*8*2048 = 65536)
GB_N = 4                     # gather buffers
STBL = 256                   # score table rows
# score table layout bases
BASES = {"hairpin": 0, "bulge": 31, "internal": 62, "sym": 93,
         "asym": 109, "helix": 138, "expl": 169, "b": 194}

_NC_CACHE = {}


def build_program(reps: int = 1, tables: bool = True, use_ttr: bool = False, finals: bool = True):
    A = mybir.AluOpType
    nc = bacc.Bacc("TRN2", target_bir_lowering=False, debug=False,
                   num_devices=N_CORES)
    etbl_d = nc.dram_tensor("etbl", [128, L], mybir.dt.float32,
                            kind="ExternalInput")
    pidx_d = nc.dram_tensor("pidx", [128, NCH * CPB * 2], mybir.dt.int16,
                            kind="ExternalInput")
    tabidx_d = nc.dram_tensor("tabidx", [128, 513], mybir.dt.int16,
                              kind="ExternalInput")
    tabidx2_d = nc.dram_tensor("tabidx2", [128, 128], mybir.dt.int16,
                               kind="ExternalInput")
    limt_d = nc.dram_tensor("limt", [128, 513], mybir.dt.int16,
                            kind="ExternalInput")
    bases_d = nc.dram_tensor("bases", [128, 513], mybir.dt.int16,
                             kind="ExternalInput")
    stbl_d = nc.dram_tensor("stbl", [128, STBL], mybir.dt.float32,
                            kind="ExternalInput")
    wsb_d = nc.dram_tensor("wsb", [128, T * 64], mybir.dt.float32,
                           kind="ExternalInput")
    ones_d = nc.dram_tensor("ones", [128, 1], mybir.dt.float32,
                            kind="ExternalInput")
    out_d = nc.dram_tensor("out", [1, 1], mybir.dt.float32,
                           kind="ExternalOutput")

    from contextlib import ExitStack
    with ExitStack() as stack, nc.Block() as block:
        ec = stack.enter_context
        etbl_s = ec(nc.sbuf_tensor("etbl_s", [128, L, 1], mybir.dt.float32))
        pidx_s = ec(nc.sbuf_tensor("pidx_s", [128, NCH * CPB * 2],
                                    mybir.dt.int16))
        tabidx_s = ec(nc.sbuf_tensor("tabidx_s", [128, 513], mybir.dt.int16))
        tabidx2_s = ec(nc.sbuf_tensor("tabidx2_s", [128, 128], mybir.dt.int16))
        limt_s = ec(nc.sbuf_tensor("limt_s", [128, 513], mybir.dt.int16))
        bases_s = ec(nc.sbuf_tensor("bases_s", [128, 513], mybir.dt.int16))
        clamped = ec(nc.sbuf_tensor("clamped", [128, 513], mybir.dt.int16))
        e1min = ec(nc.sbuf_tensor("e1min", [128, 128], mybir.dt.int16))
        stbl_s = ec(nc.sbuf_tensor("stbl_s", [128, STBL, 1], mybir.dt.float32))
        wsb_s = ec(nc.sbuf_tensor("wsb_s", [128, T * 64], mybir.dt.float32))
        ones_s = ec(nc.sbuf_tensor("ones_s", [128, 1], mybir.dt.float32))
        jbufs = [ec(nc.sbuf_tensor(f"jb{i}", [128, NI, 1], mybir.dt.float32))
                 for i in range(3)]
        ibufs = [ec(nc.sbuf_tensor(f"ib{i}", [128, NI, 1], mybir.dt.float32))
                 for i in range(3)]
        tbufs = [ec(nc.sbuf_tensor(f"tb{i}", [128, TNI, 1], mybir.dt.float32))
                 for i in range(2)]
        tbufb = ec(nc.sbuf_tensor("tbufb", [128, 16, 1], mybir.dt.float32))
        scr = [ec(nc.sbuf_tensor(f"scr{i}", [64, NI], mybir.dt.float32))
               for i in range(2)]
        tscr = ec(nc.sbuf_tensor("tscr", [128, TNI], mybir.dt.float32))
        tscrb = ec(nc.sbuf_tensor("tscrb", [128, 16], mybir.dt.float32))
        pscr = ec(nc.sbuf_tensor("pscr", [64, NCH], mybir.dt.float32))
        fscr = ec(nc.sbuf_tensor("fscr", [128, 6], mybir.dt.float32))
        pacc = ec(nc.sbuf_tensor("pacc", [64, NCH], mybir.dt.float32))
        pfin = ec(nc.sbuf_tensor("pfin", [128, 6], mybir.dt.float32))
        fincol = ec(nc.sbuf_tensor("fincol", [128, 1], mybir.dt.float32))
        outbuf = ec(nc.sbuf_tensor("outbuf", [1, 1], mybir.dt.float32))
        ps = [ec(nc.psum_tensor(f"ps{i}", [64, NI], mybir.dt.float32))
              for i in range(2)]

        io = ec(nc.semaphore("io"))
        gsem = ec(nc.semaphore("gsem"))    # Pool -> PE: pair chunk gathered
        msem = ec(nc.semaphore("msem"))    # PE -> DVE: Z ready
        dsem = ec(nc.semaphore("dsem"))    # DVE -> Pool/PE: chunk consumed
        tpsem = ec(nc.semaphore("tpsem"))  # DVE -> Pool: idx prep done
        tgsem = ec(nc.semaphore("tgsem"))  # Pool -> DVE: table chunk ready
        tcsem = ec(nc.semaphore("tcsem"))  # DVE -> Pool: table chunk consumed
        fsem = ec(nc.semaphore("fsem"))    # DVE -> PE: fincol ready
        psem2 = ec(nc.semaphore("psem2"))  # PE -> DVE: scalar psum ready
        rsem = ec(nc.semaphore("rsem"))    # DVE -> SP: outbuf ready

        N_LOADS = 9

        @block.sync
        def _(sync):
            sync.dma_start(etbl_s[:, :, 0], etbl_d[:]).then_inc(io, 16)
            sync.dma_start(pidx_s[:], pidx_d[:]).then_inc(io, 16)
            sync.dma_start(tabidx_s[:], tabidx_d[:]).then_inc(io, 16)
            sync.dma_start(tabidx2_s[:], tabidx2_d[:]).then_inc(io, 16)
            sync.dma_start(limt_s[:], limt_d[:]).then_inc(io, 16)
            sync.dma_start(bases_s[:], bases_d[:]).then_inc(io, 16)
            sync.dma_start(stbl_s[:, :, 0], stbl_d[:]).then_inc(io, 16)
            sync.dma_start(wsb_s[:], wsb_d[:]).then_inc(io, 16)
            sync.dma_start(ones_s[:], ones_d[:]).then_inc(io, 16)
            for r in range(reps):
                sync.wait_ge(rsem, r + 1)
                sync.dma_start(out_d[:], outbuf[:]).then_inc(io, 16)
            sync.wait_ge(io, 16 * (N_LOADS + reps))

        @block.gpsimd
        def _(g):
            g.load_library(apg_lib)
            g.wait_ge(io, 16 * N_LOADS)
            for r in range(reps):
                for c in range(NCH):
                    gb = r * NCH + c
                    if gb >= 3:
                        g.wait_ge(dsem, gb - 2)
                    g.ap_gather(
                        jbufs[gb % 3][:], etbl_s[:],
                        pidx_s[:, 2 * c * CPB:(2 * c + 1) * CPB],
                        128, L, 1, NI,
                    ).then_inc(gsem, 1)
                    g.ap_gather(
                        ibufs[gb % 3][:], etbl_s[:],
                        pidx_s[:, (2 * c + 1) * CPB:(2 * c + 2) * CPB],
                        128, L, 1, NI,
                    ).then_inc(gsem, 1)
                if not tables:
                    continue
                g.wait_ge(tpsem, r + 1)
                for m in range(TIN):
                    tb = r * (TIN + 1) + m
                    if tb >= 2:
                        g.wait_ge(tcsem, tb - 1)
                    g.ap_gather(
                        tbufs[tb % 2][:], stbl_s[:],
                        clamped[:, m * CPB:(m + 1) * CPB],
                        128, STBL, 1, TNI,
                    ).then_inc(tgsem, 1)
                if r > 0:
                    g.wait_ge(tcsem, r * (TIN + 1))  # prev rep b consumed
                g.ap_gather(tbufb[:], stbl_s[:], clamped[:, 512:513],
                            128, STBL, 1, 16).then_inc(tgsem, 1)

        @block.tensor
        def _(pe):
            pe.wait_ge(io, 16 * N_LOADS)
            for r in range(reps):
                if r > 0:
                    pe.wait_ge(rsem, r)  # scalar psum copied out
                for c in range(NCH):
                    gb = r * NCH + c
                    t = c // (PC // NI)
                    pe.wait_ge(gsem, 2 * gb + 1)
                    if gb >= 2:
                        pe.wait_ge(dsem, gb - 1)
                    for sb in range(NI // 512):
                        pe.matmul(
                            ps[gb % 2][:, sb * 512:(sb + 1) * 512],
                            wsb_s[0:64, t * 64:(t + 1) * 64],
                            jbufs[gb % 3][0:64, sb * 512:(sb + 1) * 512, 0],
                            start=True, stop=True,
                        ).then_inc(msem, 1)
                pe.wait_ge(fsem, r + 1)
                pe.matmul(ps[0][0:1, 0:1], ones_s[:], fincol[:],
                          start=True, stop=True,
                          tile_position=(0, 0)).then_inc(psem2, 1)

        vsem = ec(nc.semaphore("vsem"))

        @block.vector
        def _(v):
            nv = [0]

            def V(inst):
                # same-engine ordering chain (the race model gives no
                # implicit ordering between DVE instructions)
                inst.then_inc(vsem, 1)
                nv[0] += 1
                v.wait_ge(vsem, nv[0])
                return inst

            v.wait_ge(io, 16 * N_LOADS)
            for r in range(reps):
                # ---- table index prep (int16) ----
                if r > 0:
                    if tables:
                        v.wait_ge(tpsem, r)
                    v.wait_ge(fsem, r)     # pfin free (read by prev fscr)
                if not tables:
                    V(v.memset(pfin[:], 0.0))
                    mset0 = nv[0]
                if tables:
                    V(v.tensor_tensor(clamped[:], tabidx_s[:], limt_s[:], A.min))
                    V(v.tensor_scalar(clamped[0:64, 384:512],
                                      clamped[0:64, 384:512], 5, None,
                                      A.mult))
                    V(v.tensor_scalar(e1min[0:64, :], tabidx2_s[0:64, :],
                                      4, None, A.min))
                    V(v.tensor_tensor(clamped[0:64, 384:512],
                                      clamped[0:64, 384:512], e1min[0:64, :],
                                      A.add))
                    v.tensor_tensor(clamped[:], clamped[:], bases_s[:],
                                    A.add).then_inc(tpsem, 1)
                    v.wait_ge(tpsem, r + 1)
                    V(v.memset(pfin[:], 0.0))
                    mset = nv[0]
                # ---- pair-chunk dots ----
                for c in range(NCH):
                    gb = r * NCH + c
                    v.wait_ge(msem, (NI // 512) * (gb + 1))
                    v.wait_ge(gsem, 2 * gb + 2)
                    if use_ttr:
                        v.tensor_tensor_reduce(
                            scr[gb % 2][:], ps[gb % 2][:],
                            ibufs[gb % 3][0:64, :, 0],
                            1.0, 0.0, A.mult, A.add,
                            accum_out=pacc[:, c:c + 1],
                        ).then_inc(dsem, 1)
                    else:
                        V(v.tensor_tensor(scr[gb % 2][:], ps[gb % 2][:],
                                          ibufs[gb % 3][0:64, :, 0], A.mult))
                        v.tensor_scalar(scr[gb % 2][:], scr[gb % 2][:],
                                        1.0, 0.0, A.mult, A.add,
                                        accum_out=pacc[:, c:c + 1],
                                        ).then_inc(dsem, 1)
                # ---- table accums ----
                for m in (range(TIN) if tables else []):
                    tb = r * (TIN + 1) + m
                    v.wait_ge(tgsem, tb + 1)
                    v.wait_ge(vsem, mset)  # after pfin memset
                    if tb > 0:
                        v.wait_ge(tcsem, tb)   # tscr WAW chain
                    v.tensor_scalar(tscr[:], tbufs[tb % 2][:, :, 0],
                                    1.0 / 16.0, 0.0, A.mult, A.add,
                                    accum_out=pfin[:, 1 + m:2 + m],
                                    ).then_inc(tcsem, 1)
                if tables:
                    v.wait_ge(tgsem, r * (TIN + 1) + TIN + 1)
                    v.wait_ge(tcsem, r * (TIN + 1) + TIN)
                    v.tensor_scalar(tscrb[:], tbufb[:, :, 0],
                                    float(PC) / 128.0, 0.0,
                                    A.mult, A.add,
                                    accum_out=pfin[:, 5:6]).then_inc(tcsem, 1)
                # ---- finals ----
                v.wait_ge(dsem, (r + 1) * NCH)   # all pacc columns written
                V(v.tensor_scalar(pscr[:], pacc[:], 1.0, 0.0, A.mult, A.add,
                                  accum_out=pfin[0:64, 0:1]))
                if tables:
                    v.wait_ge(tcsem, (r + 1) * (TIN + 1))
                v.tensor_scalar(fscr[:], pfin[:], 1.0, 0.0, A.mult, A.add,
                                accum_out=fincol[:]).then_inc(fsem, 1)
                v.wait_ge(psem2, r + 1)
                if r > 0:
                    v.wait_ge(io, 16 * (N_LOADS + r))
                v.tensor_copy(outbuf[:], ps[0][0:1, 0:1]).then_inc(rsem, 1)
                v.wait_ge(rsem, r + 1)

    nc.compile()
    return nc


def _get_nc(reps: int = 1, **feat):
    key = (reps, tuple(sorted(feat.items())))
    if key not in _NC_CACHE:
        _NC_CACHE[key] = build_program(reps, **feat)
    return _NC_CACHE[key]


def _wrap16(a):
    # [N] int -> [16, N/16] wrapped layout (idx k at row k%16, col k//16)
    return np.ascontiguousarray(a.reshape(-1, 16).T.astype(np.int16))


def make_in_maps(inputs: dict) -> list[dict]:
    emb = np.asarray(inputs["embedding"], np.float32)
    W = np.asarray(inputs["W"], np.float32)
    b = np.asarray(inputs["b"], np.float32)
    pair_idx = np.asarray(inputs["pair_idx"], np.int32)
    explicit = np.asarray(inputs["explicit_idx"], np.int32)

    # E^T columns per partition, dims replicated across halves
    etbl = np.ascontiguousarray(np.tile(emb.T, (2, 1)))  # [128, L]

    # W blocks on partitions 0-63: wsb[d, t*64+d'] = W[t, d', d]
    wsb = np.zeros((128, T * 64), np.float32)
    wsb[0:64, :] = W.transpose(0, 2, 1).transpose(1, 0, 2).reshape(64, T * 64)

    # score table row (per partition, replicated)
    srow = np.zeros(STBL, np.float32)
    srow[0:31] = np.asarray(inputs["hairpin_length"], np.float32)
    srow[31:62] = np.asarray(inputs["bulge_length"], np.float32)
    srow[62:93] = np.asarray(inputs["internal_length"], np.float32)
    srow[93:109] = np.asarray(inputs["internal_symmetry"], np.float32)
    srow[109:138] = np.asarray(inputs["internal_asymmetry"], np.float32)
    srow[138:169] = np.asarray(inputs["helix_length"], np.float32)
    srow[169:194] = np.asarray(inputs["internal_explicit"],
                               np.float32).reshape(25)
    srow[194:203] = b
    stbl = np.ascontiguousarray(np.tile(srow[None, :], (128, 1)))

    ones = np.ones((128, 1), np.float32)

    # per-position limits and bases for the flat table-idx stream
    tab_specs = [("hairpin_idx", 30, 0), ("bulge_idx", 30, 31),
                 ("internal_len_idx", 30, 62), ("symmetry_idx", 15, 93),
                 ("asymmetry_idx", 28, 109), ("helix_idx", 30, 138)]

    in_maps = []
    for core in range(N_CORES):
        # ---- pair idx blocks ----
        pi = pair_idx[:, core * PC:(core + 1) * PC, :]  # [T, PC, 2]
        pidx = np.zeros((128, NCH * CPB * 2), np.int16)
        for c in range(NCH):
            t, s = divmod(c, PC // NI)
            seg = pi[t, s * NI:(s + 1) * NI]
            wi = _wrap16(seg[:, 0])
            wj = _wrap16(seg[:, 1])
            pidx[:, 2 * c * CPB:(2 * c + 1) * CPB] = np.tile(wj, (8, 1))
            pidx[:, (2 * c + 1) * CPB:(2 * c + 2) * CPB] = np.tile(wi, (8, 1))

        # ---- table idx stream: 7 tables x QC + pad to 4*16384 ----
        streams, lims, bass_ = [], [], []
        for name, lim, base in tab_specs:
            arr = np.asarray(inputs[name], np.int32)[core * QC:(core + 1) * QC]
            streams.append(arr)
            lims.append(np.full(QC, lim, np.int32))
            bass_.append(np.full(QC, base, np.int32))
        e0 = explicit[core * QC:(core + 1) * QC, 0]
        e1 = explicit[core * QC:(core + 1) * QC, 1]
        streams.append(e0)
        lims.append(np.full(QC, 4, np.int32))
        bass_.append(np.full(QC, 169, np.int32))
        pad_n = TIN * 8 * TNI - 7 * QC
        streams.append(np.full(pad_n, 255, np.int32))
        lims.append(np.full(pad_n, 255, np.int32))
        bass_.append(np.full(pad_n, 0, np.int32))
        stream = np.concatenate(streams)
        limst = np.concatenate(lims)
        basst = np.concatenate(bass_)

        def layout(st):
            # k = m*16384 + g*2048 + w*16 + q -> [16g+q, 128m+w]
            a = st.reshape(TIN, 8, CPB, 16)
            outm = np.zeros((128, TIN * CPB), st.dtype)
            for m in range(TIN):
                for gg in range(8):
                    outm[16 * gg:16 * gg + 16, CPB * m:CPB * (m + 1)] = \
                        a[m, gg].T
            return outm

        tabidx = np.zeros((128, 513), np.int16)
        tabidx[:, 0:512] = layout(stream).astype(np.int16)
        limt = np.zeros((128, 513), np.int16)
        limt[:, 0:512] = layout(limst).astype(np.int16)
        basesm = np.zeros((128, 513), np.int16)
        basesm[:, 0:512] = layout(basst).astype(np.int16)
        # b column (col 512): idx 194+q for q<9 else 255 in every group
        bcol = np.full(16, 255, np.int16)
        bcol[0:9] = 194 + np.arange(9, dtype=np.int16)
        tabidx[:, 512] = np.tile(bcol, 8)
        limt[:, 512] = 255
        basesm[:, 512] = 0

        # e1 aligned with the expl region (instr 3, groups 0-3)
        tabidx2 = np.zeros((128, 128), np.int16)
        a = e1.astype(np.int16).reshape(4, CPB, 16)
        for gg in range(4):
            tabidx2[16 * gg:16 * gg + 16, :] = a[gg].T

        in_maps.append({
            "etbl": etbl, "pidx": np.ascontiguousarray(pidx),
            "tabidx": np.ascontiguousarray(tabidx),
            "tabidx2": np.ascontiguousarray(tabidx2),
            "limt": np.ascontiguousarray(limt),
            "bases": np.ascontiguousarray(basesm),
            "stbl": stbl, "wsb": wsb, "ones": ones,
        })
    return in_maps


def run(in_maps, reps: int = 1, **feat):
    nc = _get_nc(reps, **feat)
    return run_bass_kernel_spmd(nc, in_maps, list(range(N_CORES)))


def kernel(**inputs) -> np.ndarray:
    in_maps = make_in_maps(inputs)
    res = run(in_maps, reps=1)
    total = np.float64(0.0)
    for c in range(N_CORES):
        total += np.float64(res.results[c]["out"].reshape(()))
    return np.array(total, dtype=np.float32)


# revision 3
# speedup vs baseline: 2.1562x; 1.8562x over previous
"""Trainium2 Bass kernel for nn_PositionalScore — ap_gather design.

Per core per rep (8-way data parallel over pairs / table indices):
  - E^T lives in SBUF as per-partition columns: partition p holds E[:, p%64]
    (dims replicated across the two 64-partition halves).
  - One gpsimd ap_gather per 2048-pair chunk fetches BOTH sides in [d, pair]
    layout: groups 0-3 (partitions 0-63) gather e_i columns, groups 4-7
    (partitions 64-127) gather e_j columns.
  - PE: Z = W_t^T @ Ej^T per chunk (lhsT/rhs at partition base 64,
    tile_position=(64,0)), accumulated nowhere — each chunk's Z is dotted
    immediately.
  - DVE: <Ei^T, Z> per chunk via tensor_tensor_reduce -> pacc column.
  - Score tables: clamped-index lookups via 4+1 more ap_gathers from a
    per-partition score table; index clamp/combine on DVE int16.
  - Final: per-partition partials -> ones-matmul over 128 partitions ->
    scalar; host sums the 8 per-core scalars.
"""

import numpy as np

import concourse.bass as bass  # noqa: F401
import concourse.bacc as bacc
from concourse import mybir
from concourse.bass_utils import run_bass_kernel_spmd
from concourse.library_config import ap_gather as apg_lib

L, D, T, P, Q = 8192, 64, 9, 131072, 65536
N_CORES = 8
PC = P // N_CORES            # pairs per core per t (16384)
QC = Q // N_CORES            # table idxs per core per table (8192)
NI = 2048                    # pairs per ap_gather chunk
NCH = T * PC // NI           # 72 chunks per rep (8 per t)
CPB = NI // 16               # idx cols per chunk (128)
TNI = 2048                   # lookups per group per table-gather
TIN = 4                      # table gather instructions (4*8*2048 = 65536)
GB_N = 4                     # gather buffers
STBL = 256                   # score table rows
# score table layout bases
BASES = {"hairpin": 0, "bulge": 31, "internal": 62, "sym": 93,
         "asym": 109, "helix": 138, "expl": 169, "b": 194}

_NC_CACHE = {}


def build_program(reps: int = 1, tables: bool = True, use_ttr: bool = False, finals: bool = True):
    A = mybir.AluOpType
    nc = bacc.Bacc("TRN2", target_bir_lowering=False, debug=False,
                   num_devices=N_CORES)
    etbl_d = nc.dram_tensor("etbl", [128, L], mybir.dt.float32,
                            kind="ExternalInput")
    pidx_d = nc.dram_tensor("pidx", [128, NCH * CPB * 2], mybir.dt.int16,
                            kind="ExternalInput")
    tabidx_d = nc.dram_tensor("tabidx", [128, 513], mybir.dt.int16,
                              kind="ExternalInput")
    tabidx2_d = nc.dram_tensor("tabidx2", [128, 128], mybir.dt.int16,
                               kind="ExternalInput")
    limt_d = nc.dram_tensor("limt", [128, 513], mybir.dt.int16,
                            kind="ExternalInput")
    bases_d = nc.dram_tensor("bases", [128, 513], mybir.dt.int16,
                             kind="ExternalInput")
    stbl_d = nc.dram_tensor("stbl", [128, STBL], mybir.dt.float32,
                            kind="ExternalInput")
    wsb_d = nc.dram_tensor("wsb", [128, T * 64], mybir.dt.float32,
                           kind="ExternalInput")
    ones_d = nc.dram_tensor("ones", [128, 1], mybir.dt.float32,
                            kind="ExternalInput")
    out_d = nc.dram_tensor("out", [1, 1], mybir.dt.float32,
                           kind="ExternalOutput")

    from contextlib import ExitStack
    with ExitStack() as stack, nc.Block() as block:
        ec = stack.enter_context
        etbl_s = ec(nc.sbuf_tensor("etbl_s", [128, L, 1], mybir.dt.float32))
        pidx_s = ec(nc.sbuf_tensor("pidx_s", [128, NCH * CPB * 2],
                                    mybir.dt.int16))
        tabidx_s = ec(nc.sbuf_tensor("tabidx_s", [128, 513], mybir.dt.int16))
        tabidx2_s = ec(nc.sbuf_tensor("tabidx2_s", [128, 128], mybir.dt.int16))
        limt_s = ec(nc.sbuf_tensor("limt_s", [128, 513], mybir.dt.int16))
        bases_s = ec(nc.sbuf_tensor("bases_s", [128, 513], mybir.dt.int16))
        clamped = ec(nc.sbuf_tensor("clamped", [128, 513], mybir.dt.int16))
        e1min = ec(nc.sbuf_tensor("e1min", [128, 128], mybir.dt.int16))
        stbl_s = ec(nc.sbuf_tensor("stbl_s", [128, STBL, 1], mybir.dt.float32))
        wsb_s = ec(nc.sbuf_tensor("wsb_s", [128, T * 64], mybir.dt.float32))
        ones_s = ec(nc.sbuf_tensor("ones_s", [128, 1], mybir.dt.float32))
        jball = ec(nc.sbuf_tensor("jball", [128, 2, NI, 1],
                                   mybir.dt.float32))
        iball = ec(nc.sbuf_tensor("iball", [128, 2, NI, 1],
                                   mybir.dt.float32))
        tbufs = [ec(nc.sbuf_tensor(f"tb{i}", [128, TNI, 1], mybir.dt.float32))
                 for i in range(2)]
        tbufb = ec(nc.sbuf_tensor("tbufb", [128, 16, 1], mybir.dt.float32))
        scr = ec(nc.sbuf_tensor("scr", [64, 2, NI], mybir.dt.float32))
        tscr = ec(nc.sbuf_tensor("tscr", [128, TNI], mybir.dt.float32))
        tscrb = ec(nc.sbuf_tensor("tscrb", [128, 16], mybir.dt.float32))
        pscr = ec(nc.sbuf_tensor("pscr", [64, NCH // 2], mybir.dt.float32))
        fscr = ec(nc.sbuf_tensor("fscr", [128, 6], mybir.dt.float32))
        pacc = ec(nc.sbuf_tensor("pacc", [64, NCH // 2], mybir.dt.float32))
        pfin = ec(nc.sbuf_tensor("pfin", [128, 6], mybir.dt.float32))
        fincol = ec(nc.sbuf_tensor("fincol", [128, 1], mybir.dt.float32))
        outbuf = ec(nc.sbuf_tensor("outbuf", [1, 1], mybir.dt.float32))
        zs = ec(nc.psum_tensor("zs", [64, 2, NI], mybir.dt.float32))

        io = ec(nc.semaphore("io"))
        gsem = ec(nc.semaphore("gsem"))    # Pool -> PE: pair chunk gathered
        msem = ec(nc.semaphore("msem"))    # PE -> DVE: Z ready
        dsem = ec(nc.semaphore("dsem"))    # DVE -> Pool/PE: chunk consumed
        tpsem = ec(nc.semaphore("tpsem"))  # DVE -> Pool: idx prep done
        tgsem = ec(nc.semaphore("tgsem"))  # Pool -> DVE: table chunk ready
        tcsem = ec(nc.semaphore("tcsem"))  # DVE -> Pool: table chunk consumed
        fsem = ec(nc.semaphore("fsem"))    # DVE -> PE: fincol ready
        psem2 = ec(nc.semaphore("psem2"))  # PE -> DVE: scalar psum ready
        rsem = ec(nc.semaphore("rsem"))    # DVE -> SP: outbuf ready

        N_LOADS = 9

        @block.sync
        def _(sync):
            sync.dma_start(etbl_s[:, :, 0], etbl_d[:]).then_inc(io, 16)
            sync.dma_start(pidx_s[:], pidx_d[:]).then_inc(io, 16)
            sync.dma_start(tabidx_s[:], tabidx_d[:]).then_inc(io, 16)
            sync.dma_start(tabidx2_s[:], tabidx2_d[:]).then_inc(io, 16)
            sync.dma_start(limt_s[:], limt_d[:]).then_inc(io, 16)
            sync.dma_start(bases_s[:], bases_d[:]).then_inc(io, 16)
            sync.dma_start(stbl_s[:, :, 0], stbl_d[:]).then_inc(io, 16)
            sync.dma_start(wsb_s[:], wsb_d[:]).then_inc(io, 16)
            sync.dma_start(ones_s[:], ones_d[:]).then_inc(io, 16)
            for r in range(reps):
                sync.wait_ge(rsem, r + 1)
                sync.dma_start(out_d[:], outbuf[:]).then_inc(io, 16)
            sync.wait_ge(io, 16 * (N_LOADS + reps))

        @block.gpsimd
        def _(g):
            g.load_library(apg_lib)
            g.wait_ge(io, 16 * N_LOADS)
            for r in range(reps):
                for c in range(NCH):
                    gb = r * NCH + c
                    if gb >= 2:
                        # slot gb%2 (and zs slot) freed once dot round
                        # gb//2 - 1 completed; also guards PE psum reuse
                        # transitively through gsem.
                        g.wait_ge(dsem, r * (NCH // 2) + c // 2)
                    g.ap_gather(
                        jball[:, gb % 2], etbl_s[:],
                        pidx_s[:, 2 * c * CPB:(2 * c + 1) * CPB],
                        128, L, 1, NI,
                    ).then_inc(gsem, 1)
                    g.ap_gather(
                        iball[:, gb % 2], etbl_s[:],
                        pidx_s[:, (2 * c + 1) * CPB:(2 * c + 2) * CPB],
                        128, L, 1, NI,
                    ).then_inc(gsem, 1)
                if not tables:
                    continue
                g.wait_ge(tpsem, r + 1)
                for m in range(TIN):
                    tb = r * (TIN + 1) + m
                    if tb >= 2:
                        g.wait_ge(tcsem, tb - 1)
                    g.ap_gather(
                        tbufs[tb % 2][:], stbl_s[:],
                        clamped[:, m * CPB:(m + 1) * CPB],
                        128, STBL, 1, TNI,
                    ).then_inc(tgsem, 1)
                if r > 0:
                    g.wait_ge(tcsem, r * (TIN + 1))  # prev rep b consumed
                g.ap_gather(tbufb[:], stbl_s[:], clamped[:, 512:513],
                            128, STBL, 1, 16).then_inc(tgsem, 1)

        @block.tensor
        def _(pe):
            pe.wait_ge(io, 16 * N_LOADS)
            for r in range(reps):
                if r > 0:
                    pe.wait_ge(rsem, r)  # scalar psum copied out
                for c in range(NCH):
                    gb = r * NCH + c
                    t = c // (PC // NI)
                    pe.wait_ge(gsem, 2 * gb + 1)
                    for sb in range(NI // 512):
                        pe.matmul(
                            zs[:, gb % 2, sb * 512:(sb + 1) * 512],
                            wsb_s[0:64, t * 64:(t + 1) * 64],
                            jball[0:64, gb % 2, sb * 512:(sb + 1) * 512, 0],
                            start=True, stop=True,
                        ).then_inc(msem, 1)
                pe.wait_ge(fsem, r + 1)
                pe.matmul(zs[0:1, 0, 0:1], ones_s[:], fincol[:],
                          start=True, stop=True,
                          tile_position=(0, 0)).then_inc(psem2, 1)

        vsem = ec(nc.semaphore("vsem"))

        @block.vector
        def _(v):
            nv = [0]

            def V(inst):
                # same-engine ordering chain (the race model gives no
                # implicit ordering between DVE instructions)
                inst.then_inc(vsem, 1)
                nv[0] += 1
                v.wait_ge(vsem, nv[0])
                return inst

            v.wait_ge(io, 16 * N_LOADS)
            for r in range(reps):
                # ---- table index prep (int16) ----
                if r > 0:
                    if tables:
                        v.wait_ge(tpsem, r)
                    v.wait_ge(fsem, r)     # pfin free (read by prev fscr)
                if not tables:
                    V(v.memset(pfin[:], 0.0))
                    mset0 = nv[0]
                if tables:
                    V(v.tensor_tensor(clamped[:], tabidx_s[:], limt_s[:], A.min))
                    V(v.tensor_scalar(clamped[0:64, 384:512],
                                      clamped[0:64, 384:512], 5, None,
                                      A.mult))
                    V(v.tensor_scalar(e1min[0:64, :], tabidx2_s[0:64, :],
                                      4, None, A.min))
                    V(v.tensor_tensor(clamped[0:64, 384:512],
                                      clamped[0:64, 384:512], e1min[0:64, :],
                                      A.add))
                    v.tensor_tensor(clamped[:], clamped[:], bases_s[:],
                                    A.add).then_inc(tpsem, 1)
                    v.wait_ge(tpsem, r + 1)
                    V(v.memset(pfin[:], 0.0))
                    mset = nv[0]
                # ---- pair dots: one per 2-chunk round ----
                for k in range(NCH // 2):
                    gb2 = r * NCH + 2 * k + 1     # second chunk of round
                    v.wait_ge(msem, (NI // 512) * (gb2 + 1))
                    v.wait_ge(gsem, 2 * gb2 + 2)  # i-gathers of both chunks
                    V(v.tensor_tensor(scr[:], zs[:],
                                      iball[0:64, :, :, 0], A.mult))
                    v.tensor_scalar(scr[:], scr[:],
                                    1.0, 0.0, A.mult, A.add,
                                    accum_out=pacc[:, k:k + 1],
                                    ).then_inc(dsem, 1)
                # ---- table accums ----
                for m in (range(TIN) if tables else []):
                    tb = r * (TIN + 1) + m
                    v.wait_ge(tgsem, tb + 1)
                    v.wait_ge(vsem, mset)  # after pfin memset
                    if tb > 0:
                        v.wait_ge(tcsem, tb)   # tscr WAW chain
                    v.tensor_scalar(tscr[:], tbufs[tb % 2][:, :, 0],
                                    1.0 / 16.0, 0.0, A.mult, A.add,
                                    accum_out=pfin[:, 1 + m:2 + m],
                                    ).then_inc(tcsem, 1)
                if tables:
                    v.wait_ge(tgsem, r * (TIN + 1) + TIN + 1)
                    v.wait_ge(tcsem, r * (TIN + 1) + TIN)
                    v.tensor_scalar(tscrb[:], tbufb[:, :, 0],
                                    float(PC) / 128.0, 0.0,
                                    A.mult, A.add,
                                    accum_out=pfin[:, 5:6]).then_inc(tcsem, 1)
                # ---- finals ----
                v.wait_ge(dsem, (r + 1) * (NCH // 2))  # all pacc written
                V(v.tensor_scalar(pscr[:], pacc[:], 1.0, 0.0, A.mult, A.add,
                                  accum_out=pfin[0:64, 0:1]))
                if tables:
                    v.wait_ge(tcsem, (r + 1) * (TIN + 1))
                v.tensor_scalar(fscr[:], pfin[:], 1.0, 0.0, A.mult, A.add,
                                accum_out=fincol[:]).then_inc(fsem, 1)
                v.wait_ge(psem2, r + 1)
                if r > 0:
                    v.wait_ge(io, 16 * (N_LOADS + r))
                v.tensor_copy(outbuf[:], zs[0:1, 0, 0:1]).then_inc(rsem, 1)
                v.wait_ge(rsem, r + 1)

    nc.compile()
    return nc


def _get_nc(reps: int = 1, **feat):
    key = (reps, tuple(sorted(feat.items())))
    if key not in _NC_CACHE:
        _NC_CACHE[key] = build_program(reps, **feat)
    return _NC_CACHE[key]


def _wrap16(a):
    # [N] int -> [16, N/16] wrapped layout (idx k at row k%16, col k//16)
    return np.ascontiguousarray(a.reshape(-1, 16).T.astype(np.int16))


def make_in_maps(inputs: dict) -> list[dict]:
    emb = np.asarray(inputs["embedding"], np.float32)
    W = np.asarray(inputs["W"], np.float32)
    b = np.asarray(inputs["b"], np.float32)
    pair_idx = np.asarray(inputs["pair_idx"], np.int32)
    explicit = np.asarray(inputs["explicit_idx"], np.int32)

    # E^T columns per partition, dims replicated across halves
    etbl = np.ascontiguousarray(np.tile(emb.T, (2, 1)))  # [128, L]

    # W blocks on partitions 0-63: wsb[d, t*64+d'] = W[t, d', d]
    wsb = np.zeros((128, T * 64), np.float32)
    wsb[0:64, :] = W.transpose(0, 2, 1).transpose(1, 0, 2).reshape(64, T * 64)

    # score table row (per partition, replicated)
    srow = np.zeros(STBL, np.float32)
    srow[0:31] = np.asarray(inputs["hairpin_length"], np.float32)
    srow[31:62] = np.asarray(inputs["bulge_length"], np.float32)
    srow[62:93] = np.asarray(inputs["internal_length"], np.float32)
    srow[93:109] = np.asarray(inputs["internal_symmetry"], np.float32)
    srow[109:138] = np.asarray(inputs["internal_asymmetry"], np.float32)
    srow[138:169] = np.asarray(inputs["helix_length"], np.float32)
    srow[169:194] = np.asarray(inputs["internal_explicit"],
                               np.float32).reshape(25)
    srow[194:203] = b
    stbl = np.ascontiguousarray(np.tile(srow[None, :], (128, 1)))

    ones = np.ones((128, 1), np.float32)

    # per-position limits and bases for the flat table-idx stream
    tab_specs = [("hairpin_idx", 30, 0), ("bulge_idx", 30, 31),
                 ("internal_len_idx", 30, 62), ("symmetry_idx", 15, 93),
                 ("asymmetry_idx", 28, 109), ("helix_idx", 30, 138)]

    in_maps = []
    for core in range(N_CORES):
        # ---- pair idx blocks ----
        pi = pair_idx[:, core * PC:(core + 1) * PC, :]  # [T, PC, 2]
        pidx = np.zeros((128, NCH * CPB * 2), np.int16)
        for c in range(NCH):
            t, s = divmod(c, PC // NI)
            seg = pi[t, s * NI:(s + 1) * NI]
            wi = _wrap16(seg[:, 0])
            wj = _wrap16(seg[:, 1])
            pidx[:, 2 * c * CPB:(2 * c + 1) * CPB] = np.tile(wj, (8, 1))
            pidx[:, (2 * c + 1) * CPB:(2 * c + 2) * CPB] = np.tile(wi, (8, 1))

        # ---- table idx stream: 7 tables x QC + pad to 4*16384 ----
        streams, lims, bass_ = [], [], []
        for name, lim, base in tab_specs:
            arr = np.asarray(inputs[name], np.int32)[core * QC:(core + 1) * QC]
            streams.append(arr)
            lims.append(np.full(QC, lim, np.int32))
            bass_.append(np.full(QC, base, np.int32))
        e0 = explicit[core * QC:(core + 1) * QC, 0]
        e1 = explicit[core * QC:(core + 1) * QC, 1]
        streams.append(e0)
        lims.append(np.full(QC, 4, np.int32))
        bass_.append(np.full(QC, 169, np.int32))
        pad_n = TIN * 8 * TNI - 7 * QC
        streams.append(np.full(pad_n, 255, np.int32))
        lims.append(np.full(pad_n, 255, np.int32))
        bass_.append(np.full(pad_n, 0, np.int32))
        stream = np.concatenate(streams)
        limst = np.concatenate(lims)
        basst = np.concatenate(bass_)

        def layout(st):
            # k = m*16384 + g*2048 + w*16 + q -> [16g+q, 128m+w]
            a = st.reshape(TIN, 8, CPB, 16)
            outm = np.zeros((128, TIN * CPB), st.dtype)
            for m in range(TIN):
                for gg in range(8):
                    outm[16 * gg:16 * gg + 16, CPB * m:CPB * (m + 1)] = \
                        a[m, gg].T
            return outm

        tabidx = np.zeros((128, 513), np.int16)
        tabidx[:, 0:512] = layout(stream).astype(np.int16)
        limt = np.zeros((128, 513), np.int16)
        limt[:, 0:512] = layout(limst).astype(np.int16)
        basesm = np.zeros((128, 513), np.int16)
        basesm[:, 0:512] = layout(basst).astype(np.int16)
        # b column (col 512): idx 194+q for q<9 else 255 in every group
        bcol = np.full(16, 255, np.int16)
        bcol[0:9] = 194 + np.arange(9, dtype=np.int16)
        tabidx[:, 512] = np.tile(bcol, 8)
        limt[:, 512] = 255
        basesm[:, 512] = 0

        # e1 aligned with the expl region (instr 3, groups 0-3)
        tabidx2 = np.zeros((128, 128), np.int16)
        a = e1.astype(np.int16).reshape(4, CPB, 16)
        for gg in range(4):
            tabidx2[16 * gg:16 * gg + 16, :] = a[gg].T

        in_maps.append({
            "etbl": etbl, "pidx": np.ascontiguousarray(pidx),
            "tabidx": np.ascontiguousarray(tabidx),
            "tabidx2": np.ascontiguousarray(tabidx2),
            "limt": np.ascontiguousarray(limt),
            "bases": np.ascontiguousarray(basesm),
            "stbl": stbl, "wsb": wsb, "ones": ones,
        })
    return in_maps


def run(in_maps, reps: int = 1, **feat):
    nc = _get_nc(reps, **feat)
    return run_bass_kernel_spmd(nc, in_maps, list(range(N_CORES)))


def kernel(**inputs) -> np.ndarray:
    in_maps = make_in_maps(inputs)
    res = run(in_maps, reps=1)
    total = np.float64(0.0)
    for c in range(N_CORES):
        total += np.float64(res.results[c]["out"].reshape(()))
    return np.array(total, dtype=np.float32)


# revision 4
# speedup vs baseline: 5.6231x; 2.6079x over previous
"""Trainium2 Bass kernel for nn_PositionalScore — ap_gather design.

Per core per rep (8-way data parallel over pairs / table indices):
  - E^T lives in SBUF as per-partition columns: partition p holds E[:, p%64]
    (dims replicated across the two 64-partition halves).
  - One gpsimd ap_gather per 2048-pair chunk fetches BOTH sides in [d, pair]
    layout: groups 0-3 (partitions 0-63) gather e_i columns, groups 4-7
    (partitions 64-127) gather e_j columns.
  - PE: Z = W_t^T @ Ej^T per chunk (lhsT/rhs at partition base 64,
    tile_position=(64,0)), accumulated nowhere — each chunk's Z is dotted
    immediately.
  - DVE: <Ei^T, Z> per chunk via tensor_tensor_reduce -> pacc column.
  - Score tables: clamped-index lookups via 4+1 more ap_gathers from a
    per-partition score table; index clamp/combine on DVE int16.
  - Final: per-partition partials -> ones-matmul over 128 partitions ->
    scalar; host sums the 8 per-core scalars.
"""

import numpy as np

import concourse.bass as bass  # noqa: F401
import concourse.bacc as bacc
from concourse import mybir
from concourse.bass_utils import run_bass_kernel_spmd
from concourse.library_config import ap_gather as apg_lib

L, D, T, P, Q = 8192, 64, 9, 131072, 65536
N_CORES = 8
PC = P // N_CORES            # pairs per core per t (16384)
QC = Q // N_CORES            # table idxs per core per table (8192)
NI = 2048                    # pairs per ap_gather chunk
NCH = T * PC // NI           # 72 chunks per rep (8 per t)
CPB = NI // 16               # idx cols per chunk (128)
TNI = 2048                   # lookups per group per table-gather
TIN = 4                      # table gather instructions (4*8*2048 = 65536)
GB_N = 4                     # gather buffers
STBL = 256                   # score table rows
# score table layout bases
BASES = {"hairpin": 0, "bulge": 31, "internal": 62, "sym": 93,
         "asym": 109, "helix": 138, "expl": 169, "b": 194}

_NC_CACHE = {}


def build_program(reps: int = 1, tables: bool = True, use_ttr: bool = False, finals: bool = True):
    A = mybir.AluOpType
    nc = bacc.Bacc("TRN2", target_bir_lowering=False, debug=False,
                   num_devices=N_CORES)
    etbl_d = nc.dram_tensor("etbl", [128, L], mybir.dt.float32,
                            kind="ExternalInput")
    pidx_d = nc.dram_tensor("pidx", [128, NCH * CPB * 2], mybir.dt.int16,
                            kind="ExternalInput")
    tabidx_d = nc.dram_tensor("tabidx", [128, 513], mybir.dt.int16,
                              kind="ExternalInput")
    tabidx2_d = nc.dram_tensor("tabidx2", [128, 128], mybir.dt.int16,
                               kind="ExternalInput")
    limt_d = nc.dram_tensor("limt", [128, 513], mybir.dt.int16,
                            kind="ExternalInput")
    bases_d = nc.dram_tensor("bases", [128, 513], mybir.dt.int16,
                             kind="ExternalInput")
    stbl_d = nc.dram_tensor("stbl", [128, STBL], mybir.dt.float32,
                            kind="ExternalInput")
    wsb_d = nc.dram_tensor("wsb", [128, T * 64], mybir.dt.float32,
                           kind="ExternalInput")
    ones_d = nc.dram_tensor("ones", [128, 1], mybir.dt.float32,
                            kind="ExternalInput")
    out_d = nc.dram_tensor("out", [1, 1], mybir.dt.float32,
                           kind="ExternalOutput")

    from contextlib import ExitStack
    with ExitStack() as stack, nc.Block() as block:
        ec = stack.enter_context
        etbl_s = ec(nc.sbuf_tensor("etbl_s", [128, L, 1], mybir.dt.float32))
        pidx_s = ec(nc.sbuf_tensor("pidx_s", [128, NCH * CPB * 2],
                                    mybir.dt.int16))
        tabidx_s = ec(nc.sbuf_tensor("tabidx_s", [128, 513], mybir.dt.int16))
        tabidx2_s = ec(nc.sbuf_tensor("tabidx2_s", [128, 128], mybir.dt.int16))
        limt_s = ec(nc.sbuf_tensor("limt_s", [128, 513], mybir.dt.int16))
        bases_s = ec(nc.sbuf_tensor("bases_s", [128, 513], mybir.dt.int16))
        clamped = ec(nc.sbuf_tensor("clamped", [128, 513], mybir.dt.int16))
        e1min = ec(nc.sbuf_tensor("e1min", [128, 128], mybir.dt.int16))
        stbl_s = ec(nc.sbuf_tensor("stbl_s", [128, STBL, 1], mybir.dt.float32))
        wsb_s = ec(nc.sbuf_tensor("wsb_s", [128, T * 64], mybir.dt.float32))
        ones_s = ec(nc.sbuf_tensor("ones_s", [128, 1], mybir.dt.float32))
        jball = ec(nc.sbuf_tensor("jball", [128, 2, NI, 1],
                                   mybir.dt.float32))
        iball = ec(nc.sbuf_tensor("iball", [128, 2, NI, 1],
                                   mybir.dt.float32))
        tbufs = [ec(nc.sbuf_tensor(f"tb{i}", [128, TNI, 1], mybir.dt.float32))
                 for i in range(2)]
        tbufb = ec(nc.sbuf_tensor("tbufb", [128, 16, 1], mybir.dt.float32))
        scr = ec(nc.sbuf_tensor("scr", [64, 2, NI], mybir.dt.float32))
        tscr = ec(nc.sbuf_tensor("tscr", [128, TNI], mybir.dt.float32))
        tscrb = ec(nc.sbuf_tensor("tscrb", [128, 16], mybir.dt.float32))
        pscr = ec(nc.sbuf_tensor("pscr", [64, NCH // 2], mybir.dt.float32))
        fscr = ec(nc.sbuf_tensor("fscr", [128, 6], mybir.dt.float32))
        pacc = ec(nc.sbuf_tensor("pacc", [64, NCH // 2], mybir.dt.float32))
        pfin = ec(nc.sbuf_tensor("pfin", [128, 6], mybir.dt.float32))
        fincol = ec(nc.sbuf_tensor("fincol", [128, 1], mybir.dt.float32))
        outbuf = ec(nc.sbuf_tensor("outbuf", [1, 1], mybir.dt.float32))
        zs = ec(nc.psum_tensor("zs", [64, 2, NI], mybir.dt.float32))

        io = ec(nc.semaphore("io"))
        gsem = ec(nc.semaphore("gsem"))    # Pool -> PE: pair chunk gathered
        msem = ec(nc.semaphore("msem"))    # PE -> DVE: Z ready
        dsem = ec(nc.semaphore("dsem"))    # DVE -> Pool/PE: chunk consumed
        tpsem = ec(nc.semaphore("tpsem"))  # DVE -> Pool: idx prep done
        tgsem = ec(nc.semaphore("tgsem"))  # Pool -> DVE: table chunk ready
        tcsem = ec(nc.semaphore("tcsem"))  # DVE -> Pool: table chunk consumed
        fsem = ec(nc.semaphore("fsem"))    # DVE -> PE: fincol ready
        psem2 = ec(nc.semaphore("psem2"))  # PE -> DVE: scalar psum ready
        rsem = ec(nc.semaphore("rsem"))    # DVE -> SP: outbuf ready

        N_LOADS = 9

        @block.sync
        def _(sync):
            sync.dma_start(etbl_s[:, :, 0], etbl_d[:]).then_inc(io, 16)
            sync.dma_start(pidx_s[:], pidx_d[:]).then_inc(io, 16)
            sync.dma_start(tabidx_s[:], tabidx_d[:]).then_inc(io, 16)
            sync.dma_start(tabidx2_s[:], tabidx2_d[:]).then_inc(io, 16)
            sync.dma_start(limt_s[:], limt_d[:]).then_inc(io, 16)
            sync.dma_start(bases_s[:], bases_d[:]).then_inc(io, 16)
            sync.dma_start(stbl_s[:, :, 0], stbl_d[:]).then_inc(io, 16)
            sync.dma_start(wsb_s[:], wsb_d[:]).then_inc(io, 16)
            sync.dma_start(ones_s[:], ones_d[:]).then_inc(io, 16)
            for r in range(reps):
                sync.wait_ge(rsem, r + 1)
                sync.dma_start(out_d[:], outbuf[:]).then_inc(io, 16)
            sync.wait_ge(io, 16 * (N_LOADS + reps))

        @block.gpsimd
        def _(g):
            g.load_library(apg_lib)
            g.wait_ge(io, 16 * N_LOADS)
            for r in range(reps):
                for c in range(NCH):
                    gb = r * NCH + c
                    if gb >= 2:
                        # slot gb%2 (and zs slot) freed once dot round
                        # gb//2 - 1 completed; also guards PE psum reuse
                        # transitively through gsem.
                        g.wait_ge(dsem, r * (NCH // 2) + c // 2)
                    g.ap_gather(
                        iball[:, gb % 2], etbl_s[:],
                        pidx_s[:, (2 * c + 1) * CPB:(2 * c + 2) * CPB],
                        128, L, 1, NI,
                    ).then_inc(gsem, 1)
                    g.ap_gather(
                        jball[:, gb % 2], etbl_s[:],
                        pidx_s[:, 2 * c * CPB:(2 * c + 1) * CPB],
                        128, L, 1, NI,
                    ).then_inc(gsem, 1)
                if not tables:
                    continue
                g.wait_ge(tpsem, r + 1)
                for m in range(TIN):
                    tb = r * (TIN + 1) + m
                    if tb >= 2:
                        g.wait_ge(tcsem, tb - 1)
                    g.ap_gather(
                        tbufs[tb % 2][:], stbl_s[:],
                        clamped[:, m * CPB:(m + 1) * CPB],
                        128, STBL, 1, TNI,
                    ).then_inc(tgsem, 1)
                if r > 0:
                    g.wait_ge(tcsem, r * (TIN + 1))  # prev rep b consumed
                g.ap_gather(tbufb[:], stbl_s[:], clamped[:, 512:513],
                            128, STBL, 1, 16).then_inc(tgsem, 1)

        @block.tensor
        def _(pe):
            pe.wait_ge(io, 16 * N_LOADS)
            for r in range(reps):
                if r > 0:
                    pe.wait_ge(rsem, r)  # scalar psum copied out
                for c in range(NCH):
                    gb = r * NCH + c
                    t = c // (PC // NI)
                    if c % 2 == 0:
                        pe.wait_ge(gsem, 2 * (gb + 2))
                    for sb in range(NI // 512):
                        pe.matmul(
                            zs[:, gb % 2, sb * 512:(sb + 1) * 512],
                            wsb_s[0:64, t * 64:(t + 1) * 64],
                            jball[0:64, gb % 2, sb * 512:(sb + 1) * 512, 0],
                            start=True, stop=True,
                        ).then_inc(msem, 1)
                pe.wait_ge(fsem, r + 1)
                pe.matmul(zs[0:1, 0, 0:1], ones_s[:], fincol[:],
                          start=True, stop=True,
                          tile_position=(0, 0)).then_inc(psem2, 1)

        vsem = ec(nc.semaphore("vsem"))

        @block.vector
        def _(v):
            nv = [0]

            def V(inst):
                # same-engine ordering chain (the race model gives no
                # implicit ordering between DVE instructions)
                inst.then_inc(vsem, 1)
                nv[0] += 1
                v.wait_ge(vsem, nv[0])
                return inst

            v.wait_ge(io, 16 * N_LOADS)
            for r in range(reps):
                # ---- table index prep (int16) ----
                if r > 0:
                    if tables:
                        v.wait_ge(tpsem, r)
                    v.wait_ge(fsem, r)     # pfin free (read by prev fscr)
                if not tables:
                    V(v.memset(pfin[:], 0.0))
                    mset0 = nv[0]
                if tables:
                    V(v.tensor_tensor(clamped[:], tabidx_s[:], limt_s[:], A.min))
                    V(v.tensor_scalar(clamped[0:64, 384:512],
                                      clamped[0:64, 384:512], 5, None,
                                      A.mult))
                    V(v.tensor_scalar(e1min[0:64, :], tabidx2_s[0:64, :],
                                      4, None, A.min))
                    V(v.tensor_tensor(clamped[0:64, 384:512],
                                      clamped[0:64, 384:512], e1min[0:64, :],
                                      A.add))
                    v.tensor_tensor(clamped[:], clamped[:], bases_s[:],
                                    A.add).then_inc(tpsem, 1)
                    v.wait_ge(tpsem, r + 1)
                    V(v.memset(pfin[:], 0.0))
                    mset = nv[0]
                # ---- pair dots: one per 2-chunk round ----
                for k in range(NCH // 2):
                    gb2 = r * NCH + 2 * k + 1     # second chunk of round
                    v.wait_ge(msem, (NI // 512) * (gb2 + 1))
                    # iball implied: msem -> PE waited gsem >= 2*(gb+2).
                    # DVE executes in order on HW; skip the vsem interlock
                    # between the mult and its accumulating reader.
                    v.tensor_tensor(scr[:], zs[:], iball[0:64, :, :, 0],
                                    A.mult).then_inc(vsem, 1)
                    nv[0] += 1
                    v.tensor_scalar(scr[:], scr[:],
                                    1.0, 0.0, A.mult, A.add,
                                    accum_out=pacc[:, k:k + 1],
                                    ).then_inc(dsem, 1)
                # ---- table accums ----
                for m in (range(TIN) if tables else []):
                    tb = r * (TIN + 1) + m
                    v.wait_ge(tgsem, tb + 1)
                    v.wait_ge(vsem, mset)  # after pfin memset
                    if tb > 0:
                        v.wait_ge(tcsem, tb)   # tscr WAW chain
                    v.tensor_scalar(tscr[:], tbufs[tb % 2][:, :, 0],
                                    1.0 / 16.0, 0.0, A.mult, A.add,
                                    accum_out=pfin[:, 1 + m:2 + m],
                                    ).then_inc(tcsem, 1)
                if tables:
                    v.wait_ge(tgsem, r * (TIN + 1) + TIN + 1)
                    v.wait_ge(tcsem, r * (TIN + 1) + TIN)
                    v.tensor_scalar(tscrb[:], tbufb[:, :, 0],
                                    float(PC) / 128.0, 0.0,
                                    A.mult, A.add,
                                    accum_out=pfin[:, 5:6]).then_inc(tcsem, 1)
                # ---- finals ----
                v.wait_ge(dsem, (r + 1) * (NCH // 2))  # all pacc written
                V(v.tensor_scalar(pscr[:], pacc[:], 1.0, 0.0, A.mult, A.add,
                                  accum_out=pfin[0:64, 0:1]))
                if tables:
                    v.wait_ge(tcsem, (r + 1) * (TIN + 1))
                v.tensor_scalar(fscr[:], pfin[:], 1.0, 0.0, A.mult, A.add,
                                accum_out=fincol[:]).then_inc(fsem, 1)
                v.wait_ge(psem2, r + 1)
                if r > 0:
                    v.wait_ge(io, 16 * (N_LOADS + r))
                v.tensor_copy(outbuf[:], zs[0:1, 0, 0:1]).then_inc(rsem, 1)
                v.wait_ge(rsem, r + 1)

    nc.compile()
    return nc


def _get_nc(reps: int = 1, **feat):
    key = (reps, tuple(sorted(feat.items())))
    if key not in _NC_CACHE:
        _NC_CACHE[key] = build_program(reps, **feat)
    return _NC_CACHE[key]


def _wrap16(a):
    # [N] int -> [16, N/16] wrapped layout (idx k at row k%16, col k//16)
    return np.ascontiguousarray(a.reshape(-1, 16).T.astype(np.int16))


def make_in_maps(inputs: dict) -> list[dict]:
    emb = np.asarray(inputs["embedding"], np.float32)
    W = np.asarray(inputs["W"], np.float32)
    b = np.asarray(inputs["b"], np.float32)
    pair_idx = np.asarray(inputs["pair_idx"], np.int32)
    explicit = np.asarray(inputs["explicit_idx"], np.int32)

    # E^T columns per partition, dims replicated across halves
    etbl = np.ascontiguousarray(np.tile(emb.T, (2, 1)))  # [128, L]

    # W blocks on partitions 0-63: wsb[d, t*64+d'] = W[t, d', d]
    wsb = np.zeros((128, T * 64), np.float32)
    wsb[0:64, :] = W.transpose(0, 2, 1).transpose(1, 0, 2).reshape(64, T * 64)

    # score table row (per partition, replicated)
    srow = np.zeros(STBL, np.float32)
    srow[0:31] = np.asarray(inputs["hairpin_length"], np.float32)
    srow[31:62] = np.asarray(inputs["bulge_length"], np.float32)
    srow[62:93] = np.asarray(inputs["internal_length"], np.float32)
    srow[93:109] = np.asarray(inputs["internal_symmetry"], np.float32)
    srow[109:138] = np.asarray(inputs["internal_asymmetry"], np.float32)
    srow[138:169] = np.asarray(inputs["helix_length"], np.float32)
    srow[169:194] = np.asarray(inputs["internal_explicit"],
                               np.float32).reshape(25)
    srow[194:203] = b
    stbl = np.ascontiguousarray(np.tile(srow[None, :], (128, 1)))

    ones = np.ones((128, 1), np.float32)

    # per-position limits and bases for the flat table-idx stream
    tab_specs = [("hairpin_idx", 30, 0), ("bulge_idx", 30, 31),
                 ("internal_len_idx", 30, 62), ("symmetry_idx", 15, 93),
                 ("asymmetry_idx", 28, 109), ("helix_idx", 30, 138)]

    in_maps = []
    for core in range(N_CORES):
        # ---- pair idx blocks ----
        pi = pair_idx[:, core * PC:(core + 1) * PC, :]  # [T, PC, 2]
        pidx = np.zeros((128, NCH * CPB * 2), np.int16)
        for c in range(NCH):
            t, s = divmod(c, PC // NI)
            seg = pi[t, s * NI:(s + 1) * NI]
            wi = _wrap16(seg[:, 0])
            wj = _wrap16(seg[:, 1])
            pidx[:, 2 * c * CPB:(2 * c + 1) * CPB] = np.tile(wj, (8, 1))
            pidx[:, (2 * c + 1) * CPB:(2 * c + 2) * CPB] = np.tile(wi, (8, 1))

        # ---- table idx stream: 7 tables x QC + pad to 4*16384 ----
        streams, lims, bass_ = [], [], []
        for name, lim, base in tab_specs:
            arr = np.asarray(inputs[name], np.int32)[core * QC:(core + 1) * QC]
            streams.append(arr)
            lims.append(np.full(QC, lim, np.int32))
            bass_.append(np.full(QC, base, np.int32))
        e0 = explicit[core * QC:(core + 1) * QC, 0]
        e1 = explicit[core * QC:(core + 1) * QC, 1]
        streams.append(e0)
        lims.append(np.full(QC, 4, np.int32))
        bass_.append(np.full(QC, 169, np.int32))
        pad_n = TIN * 8 * TNI - 7 * QC
        streams.append(np.full(pad_n, 255, np.int32))
        lims.append(np.full(pad_n, 255, np.int32))
        bass_.append(np.full(pad_n, 0, np.int32))
        stream = np.concatenate(streams)
        limst = np.concatenate(lims)
        basst = np.concatenate(bass_)

        def layout(st):
            # k = m*16384 + g*2048 + w*16 + q -> [16g+q, 128m+w]
            a = st.reshape(TIN, 8, CPB, 16)
            outm = np.zeros((128, TIN * CPB), st.dtype)
            for m in range(TIN):
                for gg in range(8):
                    outm[16 * gg:16 * gg + 16, CPB * m:CPB * (m + 1)] = \
                        a[m, gg].T
            return outm

        tabidx = np.zeros((128, 513), np.int16)
        tabidx[:, 0:512] = layout(stream).astype(np.int16)
        limt = np.zeros((128, 513), np.int16)
        limt[:, 0:512] = layout(limst).astype(np.int16)
        basesm = np.zeros((128, 513), np.int16)
        basesm[:, 0:512] = layout(basst).astype(np.int16)
        # b column (col 512): idx 194+q for q<9 else 255 in every group
        bcol = np.full(16, 255, np.int16)
        bcol[0:9] = 194 + np.arange(9, dtype=np.int16)
        tabidx[:, 512] = np.tile(bcol, 8)
        limt[:, 512] = 255
        basesm[:, 512] = 0

        # e1 aligned with the expl region (instr 3, groups 0-3)
        tabidx2 = np.zeros((128, 128), np.int16)
        a = e1.astype(np.int16).reshape(4, CPB, 16)
        for gg in range(4):
            tabidx2[16 * gg:16 * gg + 16, :] = a[gg].T

        in_maps.append({
            "etbl": etbl, "pidx": np.ascontiguousarray(pidx),
            "tabidx": np.ascontiguousarray(tabidx),
            "tabidx2": np.ascontiguousarray(tabidx2),
            "limt": np.ascontiguousarray(limt),
            "bases": np.ascontiguousarray(basesm),
            "stbl": stbl, "wsb": wsb, "ones": ones,
        })
    return in_maps


def run(in_maps, reps: int = 1, **feat):
    nc = _get_nc(reps, **feat)
    return run_bass_kernel_spmd(nc, in_maps, list(range(N_CORES)))


def kernel(**inputs) -> np.ndarray:
    in_maps = make_in_maps(inputs)
    res = run(in_maps, reps=1)
    total = np.float64(0.0)
    for c in range(N_CORES):
        total += np.float64(res.results[c]["out"].reshape(()))
    return np.array(total, dtype=np.float32)
